# revision 76
# baseline (speedup 1.0000x reference)
"""AttentionTSSA Trainium2 kernel (v2 — bf16 pipeline, PE-roofline oriented).

Problem: B=8, N=4096, DIM=1024, H=16, D=64.
  w = (x @ Wqkv.T) viewed as (b, h, n, d)
  w_normed = w / max(||w||_n, 1e-12)           (normalize over sequence axis)
  logits[b,h,n] = temp[h] * sum_d w_normed^2
  Pi = softmax over h
  Pi_norm = Pi / (sum_n Pi + 1e-8)
  dots[b,h,d] = sum_n Pi_norm * w^2
  out = -(w * Pi) * (1 / (1 + dots))
  y = out @ Wout.T + bout

Sharding: data-parallel over batch, one batch element per NeuronCore (8 cores).

Design notes (cost-model driven):
 - Matmul cost = out-free-size cycles/instr (K-independent), so the two big
   GEMMs are ~109 us each at full clock; everything else hides under them.
 - x is transposed host-side into [128, KT, N] so stage 1 needs no PE
   transposes; one big DMA per 512-wide n-chunk.
 - All SBUF tensors bf16: w, w2 (squares), Pi broadcast.  DVE elementwise ops
   on all-SBUF bf16 run in 4x mode (~194 ns/tile vs 658).
 - rsqrt^2 of the sequence norms is folded into a dynamic [c,h] mask so the
   head-sum logits matmul consumes w2 directly (no per-chunk ACT pass).
 - Pi is broadcast h->(h,d) partitions by DMA (partition_broadcast), written
   over the w2 buffer (dead after the logits matmuls) to fit SBUF.
 - dots = sum_n (Pi_b*w)*w via two DVE 4x mults + cheap reduces (DVE/ACT
   alternating).
 - -1/(1+dots/S) is folded into WoutT (per-partition scale of the stationary
   weights), so stage 4 is a plain GEMM over q = Pi_b*w.
 - bout is added host-side; y leaves as yT [128, JT, N] f32 staged via ACT.
"""
import sys

sys.path.insert(0, "/opt/trn_rl_repo")

import numpy as np
import ml_dtypes

import concourse.bacc as bacc
import concourse.bass as bass
import concourse.mybir as mybir
import concourse.tile as tile
from concourse.alu_op_type import AluOpType

F32 = mybir.dt.float32
F32R = mybir.dt.float32r
BF16 = mybir.dt.bfloat16
ACT = mybir.ActivationFunctionType
AX = mybir.AxisListType

import os as _os

PHASES = int(_os.environ.get("K_PHASES", "3"))

B, N, DIM, H, D = 8, 4096, 1024, 16, 64
CT = DIM // 128          # 8 c-tiles (each 2 heads)
KT = DIM // 128          # 8 k-tiles
NCH = N // 512           # 8 n-chunks of 512
EPS_NORM = 1e-12
EPS_PI = 1e-8


def build_nc():
    nc = bacc.Bacc(None)

    xT_d = nc.dram_tensor("xT", [128, KT, N], BF16, kind="ExternalInput")
    wq_d = nc.dram_tensor("wq", [128, KT, DIM], BF16, kind="ExternalInput")
    wo_d = nc.dram_tensor("wo", [128, CT, DIM], BF16, kind="ExternalInput")
    temph_d = nc.dram_tensor("temph", [H, 1], F32, kind="ExternalInput")
    tempf_d = nc.dram_tensor("tempf", [H, 1], F32, kind="ExternalInput")
    maskT_d = nc.dram_tensor("maskT", [128, CT, H], BF16, kind="ExternalInput")
    ones16_d = nc.dram_tensor("ones16", [H, 1], BF16, kind="ExternalInput")
    ones1x16_d = nc.dram_tensor("ones1x16", [1, H], F32, kind="ExternalInput")
    parityM_d = nc.dram_tensor("parityM", [H, 128], F32, kind="ExternalInput")
    selH_d = nc.dram_tensor("selH", [H, 8], F32, kind="ExternalInput")
    yT_d = nc.dram_tensor("yT", [128, CT, N], BF16, kind="ExternalOutput")

    with tile.TileContext(nc) as tc:
        with (
            tc.tile_pool(name="big", bufs=1) as big,
            tc.tile_pool(name="xn", bufs=2) as xn,
            tc.tile_pool(name="qp", bufs=2) as qp,
            tc.tile_pool(name="ys", bufs=2) as ysp,
            tc.tile_pool(name="tmp", bufs=3) as tmp,
            tc.tile_pool(name="p16", bufs=3) as p16,
            tc.tile_pool(name="pb", bufs=3) as pb,
            tc.tile_pool(name="pic", bufs=2) as pic,
            tc.tile_pool(name="tw", bufs=1) as twp,
            tc.tile_pool(name="pf", bufs=1) as pf,
            tc.tile_pool(name="st", bufs=1) as st,
            tc.tile_pool(name="psA", bufs=3, space="PSUM") as psA,
            tc.tile_pool(name="psB", bufs=4, space="PSUM") as psB,
            tc.tile_pool(name="psC", bufs=1, space="PSUM") as psC,
            tc.tile_pool(name="dram", bufs=1, space="DRAM") as dram,
        ):
            # ---------------- persistent SBUF ----------------
            w_sb = big.tile([128, CT, N], BF16, tag="w")       # 64 KiB/part
            # w2 / Pi-broadcast buffer, split even/odd chunks so one chunk's
            # broadcast write never conflicts with the next chunk's w2 reads
            # (dependency tracking is conservative on big shared tiles).
            w2pi_p = [
                big.tile([128, CT, N // 2], BF16, tag=f"w2pi{par}",
                         name=f"w2pi{par}")
                for par in range(2)
            ]                                                  # 64 KiB/part
            wts = big.tile([128, KT, DIM], BF16, tag="wts")    # 16 KiB/part
            maskT = big.tile([128, CT, H], BF16, tag="maskT")
            maskR = big.tile([128, CT, H], BF16, tag="maskR")
            ones16 = big.tile([H, 1], BF16, tag="ones16")
            ones1x16 = big.tile([1, H], F32R, tag="ones1x16")
            parityM = big.tile([H, 128], F32, tag="parityM")
            selH = big.tile([H, 8], F32, tag="selH")
            temph_sb = big.tile([H, 1], F32, tag="temph")
            tempf_sb = big.tile([H, 1], F32, tag="tempf")

            # first x chunk + weights first: these gate stage-1 start, the
            # consts are not needed until stats1/stage2.  wq is split per-kt
            # so the first matmuls can start while later k-tiles stream in.
            xc0 = xn.tile([128, KT, 512], BF16, tag="xT")
            nc.sync.dma_start(out=xc0, in_=xT_d[:, :, 0:512])
            for kt in range(KT):
                nc.sync.dma_start(out=wts[:, kt], in_=wq_d[:, kt, :])
            nc.sync.dma_start(out=maskT, in_=maskT_d[:, :, :])
            nc.sync.dma_start(out=ones16, in_=ones16_d[:, :])
            nc.sync.dma_start(out=ones1x16, in_=ones1x16_d[:, :].bitcast(F32R))
            nc.sync.dma_start(out=parityM, in_=parityM_d[:, :])
            nc.sync.dma_start(out=selH, in_=selH_d[:, :])
            nc.sync.dma_start(out=temph_sb, in_=temph_d[:, :])
            nc.sync.dma_start(out=tempf_sb, in_=tempf_d[:, :])

            # stats tiles
            norm2_parts = st.tile([128, CT, NCH], F32, tag="n2p")
            # dots partials split by writing engine (ACT accum vs DVE reduce)
            dots_a = st.tile([128, 3, NCH], F32, tag="dta")
            dots_d = st.tile([128, 5, NCH], F32, tag="dtd")
            norm2_c = st.tile([128, CT], F32, tag="n2c")
            rsq2 = st.tile([128, CT], F32, tag="rsq2")
            s_parts = st.tile([H, NCH], F32, tag="sp")
            s_sum = st.tile([H, 1], F32, tag="ss")
            sinv16 = st.tile([H, 1], F32, tag="sinv")
            sinvSel = st.tile([H, 8], F32, tag="sinvsel")
            sinv_c = st.tile([128, CT], F32, tag="sc")
            dots_c = st.tile([128, CT], F32, tag="dc")
            negattn = st.tile([128, CT], F32, tag="natn")

            pi_dram_p = [
                dram.tile([H, N // 2], BF16, tag=f"pi{par}", name=f"pi{par}")
                for par in range(2)
            ]

            # ---------------- stage 1: w.T = Wqkv @ x.T ----------------
            for nn in range(NCH):
                sl = slice(nn * 512, (nn + 1) * 512)
                if nn == 0:
                    xc = xc0
                else:
                    xc = xn.tile([128, KT, 512], BF16, tag="xT")
                    nc.sync.dma_start(out=xc, in_=xT_d[:, :, sl])
                for ct in range(CT):
                    wps = psA.tile([128, 512], F32, tag="psA")
                    for kt in range(KT):
                        nc.tensor.matmul(
                            wps,
                            wts[:, kt, ct * 128:(ct + 1) * 128],
                            xc[:, kt],
                            start=(kt == 0),
                            stop=(kt == KT - 1),
                        )
                    nc.vector.tensor_copy(out=w_sb[:, ct, sl], in_=wps)
                    hsl = slice((nn // 2) * 512, (nn // 2 + 1) * 512)
                    nc.scalar.activation(
                        out=w2pi_p[nn % 2][:, ct, hsl],
                        in_=wps,
                        func=ACT.Square,
                        accum_out=norm2_parts[:, ct, nn:nn + 1],
                    )

            # ---------------- stats 1: rsqrt^2, dynamic mask, wout load ----------------
            # rsq2 = 1/max(sqrt(n2), eps)^2 == 1/max(n2, eps^2)
            nc.vector.tensor_reduce(
                out=norm2_c, in_=norm2_parts, axis=AX.X, op=AluOpType.add
            )
            nc.vector.tensor_scalar_max(out=norm2_c, in0=norm2_c,
                                        scalar1=EPS_NORM * EPS_NORM)
            nc.vector.reciprocal(out=rsq2, in_=norm2_c)
            for ct in range(CT):
                nc.vector.tensor_scalar_mul(
                    out=maskR[:, ct], in0=maskT[:, ct],
                    scalar1=rsq2[:, ct:ct + 1],
                )
            # reload the weight buffer with WoutT (dead after stage-1 matmuls)
            nc.sync.dma_start(out=wts, in_=wo_d[:, :, :])

            # ---------------- stage 2 + pass A (stage-ranked pipeline) ----------------
            # Each engine's in-order queue must see work in dependency-rank
            # order across chunks, or the head of one chunk's chain blocks the
            # next chunk's independent work.  Rank r of chunk nn is emitted at
            # iteration nn + r.
            state = [dict() for _ in range(NCH)]

            def chunk_view(nn):
                return w2pi_p[nn % 2], slice((nn // 2) * 512,
                                             (nn // 2 + 1) * 512)

            def rank0(nn):       # logits matmuls
                wt, hsl = chunk_view(nn)
                lps = psA.tile([16, 512], F32, tag="psA")
                for ct in range(CT):
                    nc.tensor.matmul(
                        lps, maskR[:, ct], wt[:, ct, hsl],
                        start=(ct == 0), stop=(ct == CT - 1),
                    )
                state[nn]["lps"] = lps

            # Chunks 0,1 broadcast sqrt(Pi) into their OWN w2 slot (w2 dead
            # after logits; pass A uses u = sPi_b*w so it never rereads w2).
            # Chunks >= 2 broadcast PLAIN Pi into slot nn-2 (same parity,
            # fully dead), so pass A is one fused ttr(w2, Pi_b) per ct and
            # stage 4's q needs a single multiply.
            def bslot(nn):
                m = nn if nn < 2 else nn - 2
                return w2pi_p[m % 2], slice((m // 2) * 512,
                                            (m // 2 + 1) * 512), m

            def rank1(nn):       # E (or Eh for sqrt chunks), colsum
                scale = temph_sb if nn < 2 else tempf_sb
                Eh = p16.tile([16, 512], BF16, tag="p16")
                nc.scalar.activation(
                    out=Eh, in_=state[nn].pop("lps"), func=ACT.Exp,
                    scale=scale[:, 0:1],
                )
                if nn < 2:
                    E = pb.tile([16, 512], BF16, tag="pb")
                    nc.scalar.activation(out=E, in_=Eh, func=ACT.Square)
                else:
                    E = Eh
                csps = psB.tile([1, 512], F32, tag="psB")
                nc.tensor.matmul(csps, ones16, E, start=True, stop=True)
                state[nn]["Eh"] = Eh
                state[nn]["csps"] = csps

            def rank2(nn):       # 1/colsum (DVE), bcast over heads (PE)
                csinv = pf.tile([1, 512], F32R, tag="pf")
                with nc.allow_low_precision(reason="f32r == f32 bit layout"):
                    nc.vector.reciprocal(out=csinv, in_=state[nn].pop("csps"))
                csb = psB.tile([16, 512], F32, tag="psB")
                nc.tensor.matmul(csb, ones1x16, csinv,
                                 start=True, stop=True)
                state[nn]["csb"] = csb

            def rank3(nn):       # Pi (or sqrt(Pi)) chunk, s_parts, DMA out
                hsl = slice((nn // 2) * 512, (nn // 2 + 1) * 512)
                csbb = pb.tile([16, 512], BF16, tag="pb")
                if nn < 2:
                    nc.scalar.activation(out=csbb, in_=state[nn].pop("csb"),
                                         func=ACT.Sqrt)
                else:
                    nc.scalar.activation(out=csbb, in_=state[nn].pop("csb"),
                                         func=ACT.Identity)
                pic_t = pic.tile([16, 512], BF16, tag="pic")
                nc.vector.tensor_tensor(
                    out=pic_t, in0=state[nn].pop("Eh"), in1=csbb,
                    op=AluOpType.mult,
                )
                sdump = pb.tile([16, 512], BF16, tag="pb")
                op0 = AluOpType.mult if nn < 2 else AluOpType.max
                if _os.environ.get('K_USE_TTR'):
                    nc.vector.tensor_tensor_reduce(
                        out=sdump, in0=pic_t, in1=pic_t, scale=1.0,
                        scalar=0.0, op0=op0, op1=AluOpType.add,
                        accum_out=s_parts[:, nn:nn + 1],
                    )
                else:
                    nc.vector.tensor_tensor(out=sdump, in0=pic_t, in1=pic_t,
                                            op=op0)
                    nc.vector.tensor_reduce(out=s_parts[:, nn:nn + 1],
                                            in_=sdump, axis=AX.X,
                                            op=AluOpType.add)
                nc.sync.dma_start(out=pi_dram_p[nn % 2][:, hsl], in_=pic_t)

            def rank4(nn):       # broadcast (sqrt)Pi into a dead w2 slot
                # issued from the ACT queue so a not-yet-satisfied wait never
                # stalls the SP DMA queue
                wt, hsl, _ = bslot(nn)
                psl = slice((nn // 2) * 512, (nn // 2 + 1) * 512)
                pid = pi_dram_p[nn % 2]
                (nc.scalar if _os.environ.get('K_ACTDMA') else nc.sync).dma_start(
                    out=wt[0:64, :, hsl],
                    in_=pid[0:H:2, psl].partition_broadcast(64),
                )
                (nc.scalar if _os.environ.get('K_ACTDMA') else nc.sync).dma_start(
                    out=wt[64:128, :, hsl],
                    in_=pid[1:H:2, psl].partition_broadcast(64),
                )

            def rank5(nn):       # pass A: dots[c] += sum_n Pi_b * w^2
                bt, bsl, _ = bslot(nn)
                wt, hsl = chunk_view(nn)
                sl = slice(nn * 512, (nn + 1) * 512)
                for ct in range(CT):
                    if nn < 2:
                        # sqrt flavor: u = sPi_b*w then sum u^2
                        u = tmp.tile([128, 512], BF16, tag="tmp")
                        ueng = (nc.gpsimd if _os.environ.get('K_POOL') else nc.vector) if ct < 3 else nc.vector
                        ueng.tensor_tensor(
                            out=u, in0=bt[:, ct, bsl], in1=w_sb[:, ct, sl],
                            op=AluOpType.mult,
                        )
                        if ct < 3:
                            tdump = tmp.tile([128, 512], BF16, tag="tmp")
                            nc.scalar.activation(
                                out=tdump, in_=u, func=ACT.Square,
                                accum_out=dots_a[:, ct, nn:nn + 1],
                            )
                        else:
                            t = tmp.tile([128, 512], BF16, tag="tmp")
                            if _os.environ.get('K_USE_TTR'):
                                nc.vector.tensor_tensor_reduce(
                                    out=t, in0=u, in1=u, scale=1.0,
                                    scalar=0.0, op0=AluOpType.mult,
                                    op1=AluOpType.add,
                                    accum_out=dots_d[:, ct - 3, nn:nn + 1],
                                )
                            else:
                                nc.vector.tensor_tensor(
                                    out=t, in0=u, in1=u, op=AluOpType.mult)
                                nc.vector.tensor_reduce(
                                    out=dots_d[:, ct - 3, nn:nn + 1], in_=t,
                                    axis=AX.X, op=AluOpType.add)
                    else:
                        # plain Pi: one fused op per ct, w2 still live
                        if ct < 3:
                            t = tmp.tile([128, 512], BF16, tag="tmp")
                            peng = (nc.gpsimd
                                    if _os.environ.get('K_POOL')
                                    else nc.vector)
                            peng.tensor_tensor(
                                out=t, in0=bt[:, ct, bsl],
                                in1=wt[:, ct, hsl], op=AluOpType.mult,
                            )
                            tdump = tmp.tile([128, 512], BF16, tag="tmp")
                            nc.scalar.activation(
                                out=tdump, in_=t, func=ACT.Identity,
                                accum_out=dots_a[:, ct, nn:nn + 1],
                            )
                        else:
                            t = tmp.tile([128, 512], BF16, tag="tmp")
                            if _os.environ.get('K_USE_TTR'):
                                nc.vector.tensor_tensor_reduce(
                                    out=t, in0=bt[:, ct, bsl],
                                    in1=wt[:, ct, hsl], scale=1.0,
                                    scalar=0.0, op0=AluOpType.mult,
                                    op1=AluOpType.add,
                                    accum_out=dots_d[:, ct - 3, nn:nn + 1],
                                )
                            else:
                                nc.vector.tensor_tensor(
                                    out=t, in0=bt[:, ct, bsl],
                                    in1=wt[:, ct, hsl], op=AluOpType.mult)
                                nc.vector.tensor_reduce(
                                    out=dots_d[:, ct - 3, nn:nn + 1], in_=t,
                                    axis=AX.X, op=AluOpType.add)

            # q for chunks 0,1 is prebuilt during the pipeline drain (their
            # sqrt(Pi) slots are recycled for later chunks' Pi broadcasts)
            qc_pre = {}

            def build_q01(k):
                bt, bsl, _ = bslot(k)
                sl = slice(k * 512, (k + 1) * 512)
                qc = qp.tile([128, CT, 512], BF16, tag="q")
                uq = twp.tile([128, CT, 512], BF16, tag="tw")
                nc.vector.tensor_tensor(
                    out=uq, in0=bt[:, :, bsl], in1=w_sb[:, :, sl],
                    op=AluOpType.mult,
                )
                nc.vector.tensor_tensor(
                    out=qc, in0=uq, in1=bt[:, :, bsl], op=AluOpType.mult,
                )
                qc_pre[k] = qc

            ranks = [rank0, rank1, rank2, rank3, rank4, rank5]
            if PHASES >= 2:
                for it in range(NCH + len(ranks) - 1):
                    # oldest chunk first: its inputs are ready, so it must
                    # sit ahead of fresher work in every engine queue
                    if it == 6:
                        build_q01(0)
                    if it == 7:
                        build_q01(1)
                    for r in range(len(ranks) - 1, -1, -1):
                        nn = it - r
                        if 0 <= nn < NCH:
                            ranks[r](nn)

            # ---------------- stats 2: negattn, fold into WoutT ----------------
            nc.vector.tensor_reduce(out=dots_c[:, 0:3], in_=dots_a,
                                    axis=AX.X, op=AluOpType.add)
            nc.vector.tensor_reduce(out=dots_c[:, 3:8], in_=dots_d,
                                    axis=AX.X, op=AluOpType.add)
            nc.vector.tensor_reduce(out=s_sum, in_=s_parts, axis=AX.X,
                                    op=AluOpType.add)
            nc.vector.tensor_scalar_add(out=s_sum, in0=s_sum, scalar1=EPS_PI)
            nc.vector.reciprocal(out=sinv16, in_=s_sum)
            nc.vector.tensor_scalar_mul(out=sinvSel, in0=selH, scalar1=sinv16)
            svp = psC.tile([128, 8], F32, tag="psC")
            nc.tensor.matmul(svp, parityM, sinvSel, start=True, stop=True)
            nc.vector.tensor_copy(out=sinv_c, in_=svp)
            nc.vector.tensor_tensor(out=negattn, in0=dots_c, in1=sinv_c,
                                    op=AluOpType.mult)
            nc.vector.tensor_scalar_add(out=negattn, in0=negattn, scalar1=1.0)
            nc.vector.reciprocal(out=negattn, in_=negattn)
            nc.vector.tensor_scalar_mul(out=negattn, in0=negattn, scalar1=-1.0)
            for ct in range(CT):
                nc.vector.tensor_scalar_mul(
                    out=wts[:, ct], in0=wts[:, ct],
                    scalar1=negattn[:, ct:ct + 1],
                )

            # ---------------- stage 4 + pass B: yT = Wout' @ (Pi_b * w) ----------------
            for nn in range(NCH if PHASES >= 3 else 0):
                sl = slice(nn * 512, (nn + 1) * 512)
                if nn < 2:
                    qc = qc_pre.pop(nn)
                else:
                    qc = qp.tile([128, CT, 512], BF16, tag="q")
                    bt, bsl, _ = bslot(nn)
                    nc.vector.tensor_tensor(
                        out=qc, in0=bt[:, :, bsl], in1=w_sb[:, :, sl],
                        op=AluOpType.mult,
                    )
                for wave in range(2):
                    ys = ysp.tile([128, 4, 512], BF16, tag="ys")
                    for j4 in range(4):
                        jsub = wave * 4 + j4
                        yps = psB.tile([128, 512], F32, tag="psB")
                        for ct in range(CT):
                            nc.tensor.matmul(
                                yps,
                                wts[:, ct, jsub * 128:(jsub + 1) * 128],
                                qc[:, ct],
                                start=(ct == 0),
                                stop=(ct == CT - 1),
                            )
                        nc.scalar.copy(out=ys[:, j4], in_=yps)
                    nc.sync.dma_start(
                        out=yT_d[:, wave * 4:(wave + 1) * 4, sl], in_=ys
                    )

    nc.finalize()
    return nc


def _merge_act_table_loads(nc):
    """All activation funcs used here (Exp, Ln, Square, Identity, Copy) live
    together in the `natural_log_exp_and_others` table set, but the insertion
    pass picks a different set per function and thrashes 1.3us reloads every
    chunk.  Point the first load at the combined set and drop the rest."""
    from concourse.hw_specs import get_activation_tables

    tabs = list(get_activation_tables(nc.m.arch).items())
    combined = next(
        i for i, (name, _) in enumerate(tabs)
        if name == "natural_log_exp_and_others"
    )
    funcs = tabs[combined][1]
    for b in nc.m.functions[0].blocks:
        for inst in b.instructions:
            if inst.opcode == "Activation":
                assert inst.func in funcs, f"{inst.func} not in combined set"
    first = True
    for b in nc.m.functions[0].blocks:
        insts = b.instructions
        keep = []
        for inst in insts:
            if inst.opcode == "LoadActFuncSet":
                if first:
                    inst.act_func_set_id = combined
                    keep.append(inst)
                    first = False
            else:
                keep.append(inst)
        if len(keep) != len(insts):
            b.instructions = keep


_NC_CACHE = {}


def _get_nc():
    if "nc" not in _NC_CACHE:
        _NC_CACHE["nc"] = build_nc()
    return _NC_CACHE["nc"]


def make_host_inputs(x, Wqkv, temp, Wout, bout):
    """Per-core input maps (host-side sharding, transposes, bf16 casts)."""
    BF = ml_dtypes.bfloat16
    x = np.asarray(x, dtype=np.float32)
    # xT3[p, kt, n] = x[b][n, kt*128 + p]
    wqkvT = np.asarray(Wqkv, dtype=np.float32).T          # [k, c]
    wq3 = np.ascontiguousarray(
        wqkvT.reshape(KT, 128, DIM).transpose(1, 0, 2)
    ).astype(BF)
    woutT = np.asarray(Wout, dtype=np.float32).T          # [c, j]
    wo3 = np.ascontiguousarray(
        woutT.reshape(CT, 128, DIM).transpose(1, 0, 2)
    ).astype(BF)
    tempf = np.ascontiguousarray(
        np.asarray(temp, dtype=np.float32).reshape(H, 1)
    )
    temph = np.ascontiguousarray(tempf * 0.5)
    p = np.arange(128)
    maskT = np.zeros((128, CT, H), dtype=np.float32)
    for ct in range(CT):
        maskT[p, ct, 2 * ct + (p >= 64)] = 1.0
    maskT = maskT.astype(BF)
    ones16 = np.ones((H, 1), dtype=BF)
    ones1x16 = np.ones((1, H), dtype=np.float32)
    parityM = np.zeros((H, 128), dtype=np.float32)
    for h in range(H):
        parityM[h, :] = ((np.arange(128) >= 64) == (h % 2)).astype(np.float32)
    selH = np.zeros((H, 8), dtype=np.float32)
    for h in range(H):
        selH[h, h // 2] = 1.0

    shared = {
        "wq": wq3, "wo": wo3, "temph": temph, "tempf": tempf,
        "maskT": maskT, "ones16": ones16, "ones1x16": ones1x16,
        "parityM": parityM, "selH": selH,
    }
    maps = []
    for b in range(B):
        m = dict(shared)
        xT = np.ascontiguousarray(x[b].T)                 # [DIM, N]
        m["xT"] = np.ascontiguousarray(
            xT.reshape(KT, 128, N).transpose(1, 0, 2)
        ).astype(BF)
        maps.append(m)
    return maps


def kernel(x, Wqkv, temp, Wout, bout):
    from concourse.bass_utils import run_bass_kernel_spmd

    nc = _get_nc()
    in_maps = make_host_inputs(x, Wqkv, temp, Wout, bout)
    res = run_bass_kernel_spmd(nc, in_maps, list(range(B)))
    bout = np.asarray(bout, dtype=np.float32).reshape(1, DIM)
    y = np.empty((B, N, DIM), dtype=np.float32)
    for b in range(B):
        yt3 = np.asarray(res.results[b]["yT"], dtype=np.float32)
        yt = yt3.transpose(1, 0, 2).reshape(DIM, N)        # [j, n]
        y[b] = yt.T + bout
    return y


# revision 80
# speedup vs baseline: 1.0315x; 1.0315x over previous
"""AttentionTSSA Trainium2 kernel (v2 — bf16 pipeline, PE-roofline oriented).

Problem: B=8, N=4096, DIM=1024, H=16, D=64.
  w = (x @ Wqkv.T) viewed as (b, h, n, d)
  w_normed = w / max(||w||_n, 1e-12)           (normalize over sequence axis)
  logits[b,h,n] = temp[h] * sum_d w_normed^2
  Pi = softmax over h
  Pi_norm = Pi / (sum_n Pi + 1e-8)
  dots[b,h,d] = sum_n Pi_norm * w^2
  out = -(w * Pi) * (1 / (1 + dots))
  y = out @ Wout.T + bout

Sharding: data-parallel over batch, one batch element per NeuronCore (8 cores).

Design notes (cost-model driven):
 - Matmul cost = out-free-size cycles/instr (K-independent), so the two big
   GEMMs are ~109 us each at full clock; everything else hides under them.
 - x is transposed host-side into [128, KT, N] so stage 1 needs no PE
   transposes; one big DMA per 512-wide n-chunk.
 - All SBUF tensors bf16: w, w2 (squares), Pi broadcast.  DVE elementwise ops
   on all-SBUF bf16 run in 4x mode (~194 ns/tile vs 658).
 - rsqrt^2 of the sequence norms is folded into a dynamic [c,h] mask so the
   head-sum logits matmul consumes w2 directly (no per-chunk ACT pass).
 - Pi is broadcast h->(h,d) partitions by DMA (partition_broadcast), written
   over the w2 buffer (dead after the logits matmuls) to fit SBUF.
 - dots = sum_n (Pi_b*w)*w via two DVE 4x mults + cheap reduces (DVE/ACT
   alternating).
 - -1/(1+dots/S) is folded into WoutT (per-partition scale of the stationary
   weights), so stage 4 is a plain GEMM over q = Pi_b*w.
 - bout is added host-side; y leaves as yT [128, JT, N] f32 staged via ACT.
"""
import sys

sys.path.insert(0, "/opt/trn_rl_repo")

import numpy as np
import ml_dtypes

import concourse.bacc as bacc
import concourse.bass as bass
import concourse.mybir as mybir
import concourse.tile as tile
from concourse.alu_op_type import AluOpType

F32 = mybir.dt.float32
F32R = mybir.dt.float32r
BF16 = mybir.dt.bfloat16
ACT = mybir.ActivationFunctionType
AX = mybir.AxisListType

import os as _os

PHASES = int(_os.environ.get("K_PHASES", "3"))

B, N, DIM, H, D = 8, 4096, 1024, 16, 64
CT = DIM // 128          # 8 c-tiles (each 2 heads)
KT = DIM // 128          # 8 k-tiles
NCH = N // 512           # 8 n-chunks of 512
EPS_NORM = 1e-12
EPS_PI = 1e-8


def build_nc():
    nc = bacc.Bacc(None)

    xT_d = nc.dram_tensor("xT", [128, KT, N], BF16, kind="ExternalInput")
    wq_d = nc.dram_tensor("wq", [128, KT, DIM], BF16, kind="ExternalInput")
    wo_d = nc.dram_tensor("wo", [128, CT, DIM], BF16, kind="ExternalInput")
    temph_d = nc.dram_tensor("temph", [H, 1], F32, kind="ExternalInput")
    tempf_d = nc.dram_tensor("tempf", [H, 1], F32, kind="ExternalInput")
    maskT_d = nc.dram_tensor("maskT", [128, CT, H], BF16, kind="ExternalInput")
    ones16_d = nc.dram_tensor("ones16", [H, 1], BF16, kind="ExternalInput")
    ones1x16_d = nc.dram_tensor("ones1x16", [1, H], F32, kind="ExternalInput")
    parityM_d = nc.dram_tensor("parityM", [H, 128], F32, kind="ExternalInput")
    selH_d = nc.dram_tensor("selH", [H, 8], F32, kind="ExternalInput")
    yT_d = nc.dram_tensor("yT", [128, CT, N], BF16, kind="ExternalOutput")

    with tile.TileContext(nc) as tc:
        with (
            tc.tile_pool(name="big", bufs=1) as big,
            tc.tile_pool(name="xn", bufs=2) as xn,
            tc.tile_pool(name="qp", bufs=2) as qp,
            tc.tile_pool(name="ys", bufs=2) as ysp,
            tc.tile_pool(name="tmp", bufs=3) as tmp,
            tc.tile_pool(name="p16", bufs=3) as p16,
            tc.tile_pool(name="pb", bufs=3) as pb,
            tc.tile_pool(name="pic", bufs=2) as pic,
            tc.tile_pool(name="tw", bufs=1) as twp,
            tc.tile_pool(name="pf", bufs=1) as pf,
            tc.tile_pool(name="st", bufs=1) as st,
            tc.tile_pool(name="psA", bufs=3, space="PSUM") as psA,
            tc.tile_pool(name="psB", bufs=4, space="PSUM") as psB,
            tc.tile_pool(name="psC", bufs=1, space="PSUM") as psC,
            tc.tile_pool(name="dram", bufs=1, space="DRAM") as dram,
        ):
            # ---------------- persistent SBUF ----------------
            w_sb = big.tile([128, CT, N], BF16, tag="w")       # 64 KiB/part
            # w2 / Pi-broadcast buffer, split even/odd chunks so one chunk's
            # broadcast write never conflicts with the next chunk's w2 reads
            # (dependency tracking is conservative on big shared tiles).
            w2pi_p = [
                big.tile([128, CT, N // 2], BF16, tag=f"w2pi{par}",
                         name=f"w2pi{par}")
                for par in range(2)
            ]                                                  # 64 KiB/part
            wts = big.tile([128, KT, DIM], BF16, tag="wts")    # 16 KiB/part
            maskT = big.tile([128, CT, H], BF16, tag="maskT")
            maskR = big.tile([128, CT, H], BF16, tag="maskR")
            ones16 = big.tile([H, 1], BF16, tag="ones16")
            ones1x16 = big.tile([1, H], F32R, tag="ones1x16")
            parityM = big.tile([H, 128], F32, tag="parityM")
            selH = big.tile([H, 8], F32, tag="selH")
            temph_sb = big.tile([H, 1], F32, tag="temph")
            tempf_sb = big.tile([H, 1], F32, tag="tempf")

            # first x chunk + weights first: these gate stage-1 start, the
            # consts are not needed until stats1/stage2.  wq is split per-kt
            # so the first matmuls can start while later k-tiles stream in.
            xc0 = xn.tile([128, KT, 512], BF16, tag="xT")
            nc.sync.dma_start(out=xc0, in_=xT_d[:, :, 0:512])
            for kt in range(KT):
                nc.sync.dma_start(out=wts[:, kt], in_=wq_d[:, kt, :])
            nc.sync.dma_start(out=maskT, in_=maskT_d[:, :, :])
            nc.sync.dma_start(out=ones16, in_=ones16_d[:, :])
            nc.sync.dma_start(out=ones1x16, in_=ones1x16_d[:, :].bitcast(F32R))
            nc.sync.dma_start(out=parityM, in_=parityM_d[:, :])
            nc.sync.dma_start(out=selH, in_=selH_d[:, :])
            nc.sync.dma_start(out=temph_sb, in_=temph_d[:, :])
            nc.sync.dma_start(out=tempf_sb, in_=tempf_d[:, :])

            # stats tiles
            norm2_parts = st.tile([128, CT, NCH], F32, tag="n2p")
            # dots partials split by writing engine (ACT accum vs DVE reduce)
            dots_a = st.tile([128, 4, NCH], F32, tag="dta")
            dots_d = st.tile([128, 4, NCH], F32, tag="dtd")
            norm2_c = st.tile([128, CT], F32, tag="n2c")
            rsq2 = st.tile([128, CT], F32, tag="rsq2")
            s_parts = st.tile([H, NCH], F32, tag="sp")
            s_sum = st.tile([H, 1], F32, tag="ss")
            sinv16 = st.tile([H, 1], F32, tag="sinv")
            sinvSel = st.tile([H, 8], F32, tag="sinvsel")
            sinv_c = st.tile([128, CT], F32, tag="sc")
            dots_c = st.tile([128, CT], F32, tag="dc")
            negattn = st.tile([128, CT], F32, tag="natn")

            pi_dram_p = [
                dram.tile([H, N // 2], BF16, tag=f"pi{par}", name=f"pi{par}")
                for par in range(2)
            ]

            # ---------------- stage 1: w.T = Wqkv @ x.T ----------------
            for nn in range(NCH):
                sl = slice(nn * 512, (nn + 1) * 512)
                if nn == 0:
                    xc = xc0
                else:
                    xc = xn.tile([128, KT, 512], BF16, tag="xT")
                    nc.sync.dma_start(out=xc, in_=xT_d[:, :, sl])
                for ct in range(CT):
                    wps = psA.tile([128, 512], F32, tag="psA")
                    for kt in range(KT):
                        nc.tensor.matmul(
                            wps,
                            wts[:, kt, ct * 128:(ct + 1) * 128],
                            xc[:, kt],
                            start=(kt == 0),
                            stop=(kt == KT - 1),
                        )
                    nc.vector.tensor_copy(out=w_sb[:, ct, sl], in_=wps)
                    hsl = slice((nn // 2) * 512, (nn // 2 + 1) * 512)
                    nc.scalar.activation(
                        out=w2pi_p[nn % 2][:, ct, hsl],
                        in_=wps,
                        func=ACT.Square,
                        accum_out=norm2_parts[:, ct, nn:nn + 1],
                    )

            # ---------------- stats 1: rsqrt^2, dynamic mask, wout load ----------------
            # rsq2 = 1/max(sqrt(n2), eps)^2 == 1/max(n2, eps^2)
            nc.vector.tensor_reduce(
                out=norm2_c, in_=norm2_parts, axis=AX.X, op=AluOpType.add
            )
            nc.vector.tensor_scalar_max(out=norm2_c, in0=norm2_c,
                                        scalar1=EPS_NORM * EPS_NORM)
            nc.vector.reciprocal(out=rsq2, in_=norm2_c)
            for ct in range(CT):
                nc.vector.tensor_scalar_mul(
                    out=maskR[:, ct], in0=maskT[:, ct],
                    scalar1=rsq2[:, ct:ct + 1],
                )
            # reload the weight buffer with WoutT (dead after stage-1 matmuls)
            nc.sync.dma_start(out=wts, in_=wo_d[:, :, :])

            # ---------------- stage 2 + pass A (stage-ranked pipeline) ----------------
            # Each engine's in-order queue must see work in dependency-rank
            # order across chunks, or the head of one chunk's chain blocks the
            # next chunk's independent work.  Rank r of chunk nn is emitted at
            # iteration nn + r.
            state = [dict() for _ in range(NCH)]

            def chunk_view(nn):
                return w2pi_p[nn % 2], slice((nn // 2) * 512,
                                             (nn // 2 + 1) * 512)

            def rank0(nn):       # logits matmuls
                wt, hsl = chunk_view(nn)
                lps = psA.tile([16, 512], F32, tag="psA")
                for ct in range(CT):
                    nc.tensor.matmul(
                        lps, maskR[:, ct], wt[:, ct, hsl],
                        start=(ct == 0), stop=(ct == CT - 1),
                    )
                state[nn]["lps"] = lps

            # Chunks 0,1 broadcast sqrt(Pi) into their OWN w2 slot (w2 dead
            # after logits; pass A uses u = sPi_b*w so it never rereads w2).
            # Chunks >= 2 broadcast PLAIN Pi into slot nn-2 (same parity,
            # fully dead), so pass A is one fused ttr(w2, Pi_b) per ct and
            # stage 4's q needs a single multiply.
            def bslot(nn):
                m = nn if nn < 2 else nn - 2
                return w2pi_p[m % 2], slice((m // 2) * 512,
                                            (m // 2 + 1) * 512), m

            def rank1(nn):       # E (or Eh for sqrt chunks), colsum
                scale = temph_sb if nn < 2 else tempf_sb
                Eh = p16.tile([16, 512], BF16, tag="p16")
                nc.scalar.activation(
                    out=Eh, in_=state[nn].pop("lps"), func=ACT.Exp,
                    scale=scale[:, 0:1],
                )
                if nn < 2:
                    E = pb.tile([16, 512], BF16, tag="pb")
                    nc.scalar.activation(out=E, in_=Eh, func=ACT.Square)
                else:
                    E = Eh
                csps = psB.tile([1, 512], F32, tag="psB")
                nc.tensor.matmul(csps, ones16, E, start=True, stop=True)
                state[nn]["Eh"] = Eh
                state[nn]["csps"] = csps

            def rank2(nn):       # 1/colsum (DVE), bcast over heads (PE)
                csinv = pf.tile([1, 512], F32R, tag="pf")
                with nc.allow_low_precision(reason="f32r == f32 bit layout"):
                    nc.vector.reciprocal(out=csinv, in_=state[nn].pop("csps"))
                csb = psB.tile([16, 512], F32, tag="psB")
                nc.tensor.matmul(csb, ones1x16, csinv,
                                 start=True, stop=True)
                state[nn]["csb"] = csb

            def rank3(nn):       # Pi (or sqrt(Pi)) chunk, s_parts, DMA out
                hsl = slice((nn // 2) * 512, (nn // 2 + 1) * 512)
                csbb = pb.tile([16, 512], BF16, tag="pb")
                if nn < 2:
                    nc.scalar.activation(out=csbb, in_=state[nn].pop("csb"),
                                         func=ACT.Sqrt)
                else:
                    nc.scalar.activation(out=csbb, in_=state[nn].pop("csb"),
                                         func=ACT.Identity)
                pic_t = pic.tile([16, 512], BF16, tag="pic")
                nc.vector.tensor_tensor(
                    out=pic_t, in0=state[nn].pop("Eh"), in1=csbb,
                    op=AluOpType.mult,
                )
                sdump = pb.tile([16, 512], BF16, tag="pb")
                sfunc = ACT.Square if nn < 2 else ACT.Identity
                nc.scalar.activation(
                    out=sdump, in_=pic_t, func=sfunc,
                    accum_out=s_parts[:, nn:nn + 1],
                )
                nc.sync.dma_start(out=pi_dram_p[nn % 2][:, hsl], in_=pic_t)

            def rank4(nn):       # broadcast (sqrt)Pi into a dead w2 slot
                # issued from the ACT queue so a not-yet-satisfied wait never
                # stalls the SP DMA queue
                wt, hsl, _ = bslot(nn)
                psl = slice((nn // 2) * 512, (nn // 2 + 1) * 512)
                pid = pi_dram_p[nn % 2]
                (nc.scalar if _os.environ.get('K_ACTDMA') else nc.sync).dma_start(
                    out=wt[0:64, :, hsl],
                    in_=pid[0:H:2, psl].partition_broadcast(64),
                )
                (nc.scalar if _os.environ.get('K_ACTDMA') else nc.sync).dma_start(
                    out=wt[64:128, :, hsl],
                    in_=pid[1:H:2, psl].partition_broadcast(64),
                )

            def rank5(nn):       # pass A: dots[c] += sum_n Pi_b * w^2
                bt, bsl, _ = bslot(nn)
                wt, hsl = chunk_view(nn)
                sl = slice(nn * 512, (nn + 1) * 512)
                if nn < 2:
                    # sqrt flavor: u = sPi_b*w then sum u^2
                    for ct in range(4):
                        u = tmp.tile([128, 512], BF16, tag="tmp")
                        nc.vector.tensor_tensor(
                            out=u, in0=bt[:, ct, bsl], in1=w_sb[:, ct, sl],
                            op=AluOpType.mult,
                        )
                        tdump = tmp.tile([128, 512], BF16, tag="tmp")
                        nc.scalar.activation(
                            out=tdump, in_=u, func=ACT.Square,
                            accum_out=dots_a[:, ct, nn:nn + 1],
                        )
                    ug = twp.tile([128, 4, 512], BF16, tag="t4")
                    nc.vector.tensor_tensor(
                        out=ug, in0=bt[:, 4:8, bsl], in1=w_sb[:, 4:8, sl],
                        op=AluOpType.mult,
                    )
                    tg = twp.tile([128, 4, 512], BF16, tag="t4b")
                    nc.vector.tensor_tensor(
                        out=tg, in0=ug, in1=ug, op=AluOpType.mult,
                    )
                    nc.vector.tensor_reduce(
                        out=dots_d[:, :, nn:nn + 1], in_=tg,
                        axis=AX.X, op=AluOpType.add,
                    )
                else:
                    # plain Pi: t = Pi_b * w2 (w2 still live)
                    for ct in range(4):
                        t = tmp.tile([128, 512], BF16, tag="tmp")
                        nc.vector.tensor_tensor(
                            out=t, in0=bt[:, ct, bsl],
                            in1=wt[:, ct, hsl], op=AluOpType.mult,
                        )
                        tdump = tmp.tile([128, 512], BF16, tag="tmp")
                        nc.scalar.activation(
                            out=tdump, in_=t, func=ACT.Identity,
                            accum_out=dots_a[:, ct, nn:nn + 1],
                        )
                    tg = twp.tile([128, 4, 512], BF16, tag="t4")
                    nc.vector.tensor_tensor(
                        out=tg, in0=bt[:, 4:8, bsl], in1=wt[:, 4:8, hsl],
                        op=AluOpType.mult,
                    )
                    nc.vector.tensor_reduce(
                        out=dots_d[:, :, nn:nn + 1], in_=tg,
                        axis=AX.X, op=AluOpType.add,
                    )

            # q for chunks 0,1 is prebuilt during the pipeline drain (their
            # sqrt(Pi) slots are recycled for later chunks' Pi broadcasts)
            qc_pre = {}

            def build_q01(k):
                bt, bsl, _ = bslot(k)
                sl = slice(k * 512, (k + 1) * 512)
                qc = qp.tile([128, CT, 512], BF16, tag="q")
                for h4 in (slice(0, 4), slice(4, 8)):
                    uq = twp.tile([128, 4, 512], BF16, tag="t4b")
                    nc.vector.tensor_tensor(
                        out=uq, in0=bt[:, h4, bsl], in1=w_sb[:, h4, sl],
                        op=AluOpType.mult,
                    )
                    nc.vector.tensor_tensor(
                        out=qc[:, h4], in0=uq, in1=bt[:, h4, bsl],
                        op=AluOpType.mult,
                    )
                qc_pre[k] = qc

            ranks = [rank0, rank1, rank2, rank3, rank4, rank5]
            if PHASES >= 2:
                for it in range(NCH + len(ranks) - 1):
                    # oldest chunk first: its inputs are ready, so it must
                    # sit ahead of fresher work in every engine queue
                    if it == 6:
                        build_q01(0)
                    if it == 7:
                        build_q01(1)
                    for r in range(len(ranks) - 1, -1, -1):
                        nn = it - r
                        if 0 <= nn < NCH:
                            ranks[r](nn)

            # ---------------- stats 2: negattn, fold into WoutT ----------------
            nc.vector.tensor_reduce(out=dots_c[:, 0:4], in_=dots_a,
                                    axis=AX.X, op=AluOpType.add)
            nc.vector.tensor_reduce(out=dots_c[:, 4:8], in_=dots_d,
                                    axis=AX.X, op=AluOpType.add)
            nc.vector.tensor_reduce(out=s_sum, in_=s_parts, axis=AX.X,
                                    op=AluOpType.add)
            nc.vector.tensor_scalar_add(out=s_sum, in0=s_sum, scalar1=EPS_PI)
            nc.vector.reciprocal(out=sinv16, in_=s_sum)
            nc.vector.tensor_scalar_mul(out=sinvSel, in0=selH, scalar1=sinv16)
            svp = psC.tile([128, 8], F32, tag="psC")
            nc.tensor.matmul(svp, parityM, sinvSel, start=True, stop=True)
            nc.vector.tensor_copy(out=sinv_c, in_=svp)
            nc.vector.tensor_tensor(out=negattn, in0=dots_c, in1=sinv_c,
                                    op=AluOpType.mult)
            nc.vector.tensor_scalar_add(out=negattn, in0=negattn, scalar1=1.0)
            nc.vector.reciprocal(out=negattn, in_=negattn)
            nc.vector.tensor_scalar_mul(out=negattn, in0=negattn, scalar1=-1.0)
            for ct in range(CT):
                nc.vector.tensor_scalar_mul(
                    out=wts[:, ct], in0=wts[:, ct],
                    scalar1=negattn[:, ct:ct + 1],
                )

            # ---------------- stage 4 + pass B: yT = Wout' @ (Pi_b * w) ----------------
            for nn in range(NCH if PHASES >= 3 else 0):
                sl = slice(nn * 512, (nn + 1) * 512)
                if nn < 2:
                    qc = qc_pre.pop(nn)
                else:
                    qc = qp.tile([128, CT, 512], BF16, tag="q")
                    bt, bsl, _ = bslot(nn)
                    nc.vector.tensor_tensor(
                        out=qc, in0=bt[:, :, bsl], in1=w_sb[:, :, sl],
                        op=AluOpType.mult,
                    )
                # last chunk drains in single-jsub stores so the final DMA
                # isn't waiting on four ACT copies
                wsz = 1 if nn == NCH - 1 else 4
                for wave in range(CT // wsz):
                    if wsz == 4:
                        ys = ysp.tile([128, wsz, 512], BF16, tag="ys")
                    else:
                        ys = tmp.tile([128, wsz, 512], BF16, tag="tmp")
                    for j4 in range(wsz):
                        jsub = wave * wsz + j4
                        yps = psB.tile([128, 512], F32, tag="psB")
                        for ct in range(CT):
                            nc.tensor.matmul(
                                yps,
                                wts[:, ct, jsub * 128:(jsub + 1) * 128],
                                qc[:, ct],
                                start=(ct == 0),
                                stop=(ct == CT - 1),
                            )
                        nc.scalar.copy(out=ys[:, j4], in_=yps)
                    nc.sync.dma_start(
                        out=yT_d[:, wave * wsz:(wave + 1) * wsz, sl], in_=ys
                    )

    nc.finalize()
    return nc


def _merge_act_table_loads(nc):
    """All activation funcs used here (Exp, Ln, Square, Identity, Copy) live
    together in the `natural_log_exp_and_others` table set, but the insertion
    pass picks a different set per function and thrashes 1.3us reloads every
    chunk.  Point the first load at the combined set and drop the rest."""
    from concourse.hw_specs import get_activation_tables

    tabs = list(get_activation_tables(nc.m.arch).items())
    combined = next(
        i for i, (name, _) in enumerate(tabs)
        if name == "natural_log_exp_and_others"
    )
    funcs = tabs[combined][1]
    for b in nc.m.functions[0].blocks:
        for inst in b.instructions:
            if inst.opcode == "Activation":
                assert inst.func in funcs, f"{inst.func} not in combined set"
    first = True
    for b in nc.m.functions[0].blocks:
        insts = b.instructions
        keep = []
        for inst in insts:
            if inst.opcode == "LoadActFuncSet":
                if first:
                    inst.act_func_set_id = combined
                    keep.append(inst)
                    first = False
            else:
                keep.append(inst)
        if len(keep) != len(insts):
            b.instructions = keep


_NC_CACHE = {}


def _get_nc():
    if "nc" not in _NC_CACHE:
        _NC_CACHE["nc"] = build_nc()
    return _NC_CACHE["nc"]


def make_host_inputs(x, Wqkv, temp, Wout, bout):
    """Per-core input maps (host-side sharding, transposes, bf16 casts)."""
    BF = ml_dtypes.bfloat16
    x = np.asarray(x, dtype=np.float32)
    # xT3[p, kt, n] = x[b][n, kt*128 + p]
    wqkvT = np.asarray(Wqkv, dtype=np.float32).T          # [k, c]
    wq3 = np.ascontiguousarray(
        wqkvT.reshape(KT, 128, DIM).transpose(1, 0, 2)
    ).astype(BF)
    woutT = np.asarray(Wout, dtype=np.float32).T          # [c, j]
    wo3 = np.ascontiguousarray(
        woutT.reshape(CT, 128, DIM).transpose(1, 0, 2)
    ).astype(BF)
    tempf = np.ascontiguousarray(
        np.asarray(temp, dtype=np.float32).reshape(H, 1)
    )
    temph = np.ascontiguousarray(tempf * 0.5)
    p = np.arange(128)
    maskT = np.zeros((128, CT, H), dtype=np.float32)
    for ct in range(CT):
        maskT[p, ct, 2 * ct + (p >= 64)] = 1.0
    maskT = maskT.astype(BF)
    ones16 = np.ones((H, 1), dtype=BF)
    ones1x16 = np.ones((1, H), dtype=np.float32)
    parityM = np.zeros((H, 128), dtype=np.float32)
    for h in range(H):
        parityM[h, :] = ((np.arange(128) >= 64) == (h % 2)).astype(np.float32)
    selH = np.zeros((H, 8), dtype=np.float32)
    for h in range(H):
        selH[h, h // 2] = 1.0

    shared = {
        "wq": wq3, "wo": wo3, "temph": temph, "tempf": tempf,
        "maskT": maskT, "ones16": ones16, "ones1x16": ones1x16,
        "parityM": parityM, "selH": selH,
    }
    maps = []
    for b in range(B):
        m = dict(shared)
        xT = np.ascontiguousarray(x[b].T)                 # [DIM, N]
        m["xT"] = np.ascontiguousarray(
            xT.reshape(KT, 128, N).transpose(1, 0, 2)
        ).astype(BF)
        maps.append(m)
    return maps


def kernel(x, Wqkv, temp, Wout, bout):
    from concourse.bass_utils import run_bass_kernel_spmd

    nc = _get_nc()
    in_maps = make_host_inputs(x, Wqkv, temp, Wout, bout)
    res = run_bass_kernel_spmd(nc, in_maps, list(range(B)))
    bout = np.asarray(bout, dtype=np.float32).reshape(1, DIM)
    y = np.empty((B, N, DIM), dtype=np.float32)
    for b in range(B):
        yt3 = np.asarray(res.results[b]["yT"], dtype=np.float32)
        yt = yt3.transpose(1, 0, 2).reshape(DIM, N)        # [j, n]
        y[b] = yt.T + bout
    return y
